# revision 3
# baseline (speedup 1.0000x reference)
"""DSSM (dual GRU encoder + BxB softmax similarity) on 8 Trainium2 NeuronCores.

v2: fp8e4 DoubleRow recurrent matmuls + rebalanced gate elementwise.

  - Cores 0-3: context encoder rows [256c, 256(c+1)); cores 4-7: reply
    encoder. One SPMD program; per-core in_maps.
  - Encoder state: h fp16 [128 part, 4 chunk, 256 batch] + fp8e4 shadow h8
    for the PE. Per step:
      gi  = Wih~16 . xt           (fp16 MMs, K=E+1, bias row folded, x16)
      gh  = Whh~16 . h8           (fp8e4 DoubleRow MMs: 2 k-chunks/instr)
      rz  = sigmoid((gi+gh)/16)   (one ACT op per phase over r and z banks)
      rm  = (ghn + bhhn16) * r    (DVE scalar_tensor_tensor, folds bias)
      pre = gin (+) rm            (PE identity-matmul accumulate into PSUM)
      n   = tanh(pre/16) -> h_new (ACT writes n into h_new chunks)
      h_new = n + z*(h_old - n)   (DVE fp16 2x ops, in place)
      h8_new = fp8(h_new)         (Pool/GPSIMD copy)
  - Embedding path: indirect-DMA gathers (GA-step lookahead) + DMA XBAR
    transpose to xt [E+1, 256] fp16 (no PE/PSUM involvement).
  - Phase pairing: M-phase 0 computes h chunks (0,2), phase 1 (1,3);
    DoubleRow k-pairs are (0,2) and (1,3) so pair j consumes the chunks
    phase j of the previous step produced (keeps PE from waiting).
  - Phase 2 (second kernel): each core computes a 128-row slice of
    scores = hc @ hr.T (fp16 MMs, fp32 accum) + row softmax.
"""

import numpy as np

import concourse.bass as bass
import concourse.mybir as mybir
import concourse.tile as tile
from concourse import bacc
from concourse.bass_utils import run_bass_kernel_spmd
from concourse.masks import make_identity

F16 = mybir.dt.float16
F32 = mybir.dt.float32
F8 = mybir.dt.float8e4
I32 = mybir.dt.int32
DR = mybir.MatmulPerfMode.DoubleRow
Sigmoid = mybir.ActivationFunctionType.Sigmoid
Tanh = mybir.ActivationFunctionType.Tanh

V, E, H, B, L = 50000, 100, 512, 1024, 128
NB = 256          # batch rows per core
HC = H // 128     # 4 h chunks
GA = 6            # gather lookahead (steps)
XA = 2            # xt lookahead
WS = 16.0         # weight scale folded into wih/whh/bhhn; ACT applies 1/WS


def build_encoder(l_steps=L, loop_n=None):
    nc = bacc.Bacc("TRN2", target_bir_lowering=False, debug=False)
    emb_d = nc.dram_tensor("emb", [V, E], F16, kind="ExternalInput")
    idx_d = nc.dram_tensor("idx", [128, 2 * l_steps], I32, kind="ExternalInput")
    wih_d = nc.dram_tensor("wih", [E + 1, 12, 128], F16, kind="ExternalInput")
    whh_d = nc.dram_tensor("whh", [128, 2, 2, 12, 128], F8, kind="ExternalInput")
    bhhn_d = nc.dram_tensor("bhhn", [128, 4], F32, kind="ExternalInput")
    hout_d = nc.dram_tensor("hout", [128, HC, NB], F16, kind="ExternalOutput")

    with tile.TileContext(nc) as tc:
        with (
            tc.tile_pool(name="wt", bufs=1) as wt,
            tc.tile_pool(name="grng", bufs=GA + 2) as grng,
            tc.tile_pool(name="xrng", bufs=XA + 2) as xrng,
            tc.tile_pool(name="hrng", bufs=3) as hrng,
            tc.tile_pool(name="gt", bufs=4) as gt,
            tc.tile_pool(name="psrz0", bufs=1, space="PSUM") as psrz0,
            tc.tile_pool(name="psrz1", bufs=1, space="PSUM") as psrz1,
            tc.tile_pool(name="psghn", bufs=1, space="PSUM") as psghn,
            tc.tile_pool(name="psgin", bufs=2, space="PSUM") as psgin,
        ):
            # --- constants / weights ---
            wih_t = wt.tile([E + 1, 12, 128], F16, tag="wih")
            nc.sync.dma_start(out=wih_t[:], in_=wih_d[:])
            whh_t = wt.tile([128, 2, 2, 12, 128], F8, tag="whh")
            nc.sync.dma_start(out=whh_t[:], in_=whh_d[:])
            bhhn_t = wt.tile([128, 4], F32, tag="bhhn")
            nc.sync.dma_start(out=bhhn_t[:], in_=bhhn_d[:])
            idx_t = wt.tile([128, 2 * l_steps], I32, tag="idx")
            nc.sync.dma_start(out=idx_t[:], in_=idx_d[:])
            ident = wt.tile([128, 128], F16, tag="ident")
            make_identity(nc, ident[:])

            # --- rings ---
            n_g = GA + 2
            g_ring = []
            for i in range(n_g):
                g = grng.tile([128, 2, 128], F16, tag="g", name=f"g{i}")
                nc.vector.memset(g[:, :, E:], 0.0)
                nc.vector.memset(g[:, :, E : E + 1], 1.0)
                g_ring.append(g)
            n_x = XA + 2
            xt_ring = [xrng.tile([128, 2, 128], F16, tag="xt", name=f"xt{i}")
                       for i in range(n_x)]
            h_ring, h8_ring = [], []
            for i in range(3):
                h = hrng.tile([128, HC, NB], F16, tag="h", name=f"h{i}")
                nc.vector.memset(h[:], 0.0)
                h_ring.append(h)
                h8 = hrng.tile([128, HC, NB], F8, tag="h8", name=f"h8{i}")
                nc.vector.memset(h8[:], 0.0)
                h8_ring.append(h8)

            def hpair(h, b):
                # chunks (b, 2+b) as [128, 2, NB]
                return h.rearrange("p (a b) x -> p a b x", a=2, b=2)[:, :, b, :]

            def emit_gather(s):
                g = g_ring[s % n_g]
                for hh in range(2):
                    nc.gpsimd.indirect_dma_start(
                        out=g[:, hh, :E],
                        out_offset=None,
                        in_=emb_d[:],
                        in_offset=bass.IndirectOffsetOnAxis(
                            ap=idx_t[:, 2 * s + hh : 2 * s + hh + 1], axis=0
                        ),
                    )

            def emit_xt(s):
                g = g_ring[s % n_g]
                xt = xt_ring[s % n_x]
                for hh in range(2):
                    nc.sync.dma_start_transpose(xt[:, hh, :], g[:, hh, :])

            # prologue
            for s in range(min(GA + 1, l_steps)):
                emit_gather(s)
            emit_xt(0)

            from contextlib import nullcontext
            loop_cm = tc.For_i(0, loop_n, 1) if loop_n else nullcontext()
            with loop_cm:
              for s in range(l_steps):
                h_old, h8_old = h_ring[s % 3], h8_ring[s % 3]
                h_new, h8_new = h_ring[(s + 1) % 3], h8_ring[(s + 1) % 3]
                xt = xt_ring[s % n_x]
                xt_ap = xt.rearrange("p a b -> p (a b)")  # [128, 256]

                # PSUM: per-phase rz [128,2(gate),2(chunk),NB] = 2 banks
                ps_rz0 = psrz0.tile([128, 2, 2, NB], F32, tag="rz0", name=f"rz0_{s}")
                ps_rz1 = psrz1.tile([128, 2, 2, NB], F32, tag="rz1", name=f"rz1_{s}")
                ps_rz = [ps_rz0, ps_rz1]
                ps_ghn = psghn.tile([128, 4, NB], F32, tag="ghn")
                gins, rzs, rms = {}, {}, {}

                if loop_n:
                    emit_xt((s + 1) % l_steps)
                    emit_gather((s + GA + 1) % l_steps)
                else:
                    if s + 1 < l_steps:
                        emit_xt(s + 1)
                    if s + GA + 1 < l_steps:
                        emit_gather(s + GA + 1)

                def rz_gi(ph):
                    ca, cb = ph, ph + 2
                    for gi_, gb in ((0, 0), (1, 4)):  # r rows, z rows
                        nc.tensor.matmul(ps_rz[ph][:, gi_, 0, :],
                                         wih_t[:, gb + ca, :], xt_ap[:101],
                                         start=True, stop=False)
                        nc.tensor.matmul(ps_rz[ph][:, gi_, 1, :],
                                         wih_t[:, gb + cb, :], xt_ap[:101],
                                         start=False, stop=False)

                def n_gi(ph):
                    ca, cb = ph, ph + 2
                    ps_gin = psgin.tile([128, 2, NB], F32, tag="gin",
                                        name=f"gin{ph}_{s}")
                    gins[ph] = ps_gin
                    nc.tensor.matmul(ps_gin[:, 0, :], wih_t[:, 8 + ca, :],
                                     xt_ap[:101], start=True, stop=False)
                    nc.tensor.matmul(ps_gin[:, 1, :], wih_t[:, 8 + cb, :],
                                     xt_ap[:101], start=False, stop=False)

                def rz_dr(ph, j):
                    ca, cb = ph, ph + 2
                    last = j == 1
                    for gi_, gb in ((0, 0), (1, 4)):
                        for i, c in enumerate((ca, cb)):
                            nc.tensor.matmul(
                                ps_rz[ph][:, gi_, i, :],
                                whh_t[:, j, :, gb + c, :],
                                hpair(h8_old, j),
                                start=False, stop=(last and i == 1),
                                perf_mode=DR)

                def n_dr(ph, j):
                    ca, cb = ph, ph + 2
                    for i, c in enumerate((ca, cb)):
                        nc.tensor.matmul(
                            ps_ghn[:, 2 * ph + i, :],
                            whh_t[:, j, :, 8 + c, :],
                            hpair(h8_old, j),
                            start=(j == 0 and i == 0), stop=(j == 1 and i == 1),
                            perf_mode=DR)

                def act_rz(ph):
                    rz = gt.tile([128, 2, 2, NB], F16, tag="rz_s")
                    rzs[ph] = rz
                    nc.scalar.activation(rz[:], ps_rz[ph][:], Sigmoid,
                                         scale=1.0 / WS)

                def dve_rm(ph):
                    rm = gt.tile([128, 2, NB], F16, tag="rm")
                    rms[ph] = rm
                    for i in range(2):
                        c = ph + 2 * i
                        nc.vector.scalar_tensor_tensor(
                            rm[:, i, :], ps_ghn[:, 2 * ph + i, :],
                            bhhn_t[:, c : c + 1], rzs[ph][:, 0, i, :],
                            op0=mybir.AluOpType.add, op1=mybir.AluOpType.mult)

                def pe_ident(ph):
                    for i in range(2):
                        nc.tensor.matmul(gins[ph][:, i, :], ident[:],
                                         rms[ph][:, i, :],
                                         start=False, stop=(i == 1))

                def act_n(ph):
                    nc.scalar.activation(hpair(h_new, ph), gins[ph][:], Tanh,
                                         scale=1.0 / WS)

                def dve_tail(ph):
                    hmn = gt.tile([128, 2, NB], F16, tag="hmn")
                    nc.vector.tensor_sub(hmn[:], hpair(h_old, ph),
                                         hpair(h_new, ph))
                    t = gt.tile([128, 2, NB], F16, tag="t")
                    nc.vector.tensor_mul(t[:], rzs[ph][:, 1, :, :], hmn[:])
                    nc.vector.tensor_add(hpair(h_new, ph), hpair(h_new, ph),
                                         t[:])

                def pool_h8(ph):
                    nc.gpsimd.tensor_copy(hpair(h8_new, ph), hpair(h_new, ph))

                # ---- emission order ----
                rz_gi(0)
                rz_gi(1)
                n_gi(0)
                n_gi(1)
                rz_dr(0, 0)          # needs h8(0,2) of prev step (early)
                rz_dr(1, 0)
                n_dr(0, 0)
                n_dr(1, 0)
                rz_dr(0, 1)          # needs h8(1,3) of prev step (late)
                n_dr(0, 1)
                act_rz(0)
                dve_rm(0)
                rz_dr(1, 1)
                n_dr(1, 1)
                pe_ident(0)
                act_n(0)
                act_rz(1)
                dve_rm(1)
                dve_tail(0)
                pool_h8(0)
                pe_ident(1)
                act_n(1)
                dve_tail(1)
                pool_h8(1)

            nc.sync.dma_start(out=hout_d[:], in_=h_ring[l_steps % 3][:])

    nc.compile()
    return nc


def build_scores():
    nc = bacc.Bacc("TRN2", target_bir_lowering=False, debug=False)
    hc_d = nc.dram_tensor("hc", [HC, 128, 128], F16, kind="ExternalInput")
    hr_d = nc.dram_tensor("hr", [HC, 128, B], F16, kind="ExternalInput")
    out_d = nc.dram_tensor("out", [128, B], F32, kind="ExternalOutput")

    with tile.TileContext(nc) as tc:
        with (
            tc.tile_pool(name="sb", bufs=1) as sb,
            tc.tile_pool(name="ps", bufs=1, space="PSUM") as ps,
        ):
            hc_t = sb.tile([128, HC, 128], F16, tag="hc")
            hr_t = sb.tile([128, HC, B], F16, tag="hr")
            nc.sync.dma_start(out=hc_t[:], in_=hc_d.rearrange("k p m -> p k m"))
            nc.sync.dma_start(out=hr_t[:], in_=hr_d.rearrange("k p n -> p k n"))
            ps_s = ps.tile([128, B], F32, tag="s")
            for nh in range(2):
                for kj in range(HC):
                    nc.tensor.matmul(
                        ps_s[:, nh * 512 : (nh + 1) * 512],
                        hc_t[:, kj, :],
                        hr_t[:, kj, nh * 512 : (nh + 1) * 512],
                        start=(kj == 0), stop=(kj == HC - 1),
                    )
            mx = sb.tile([128, 1], F32, tag="mx")
            nc.vector.reduce_max(mx[:], ps_s[:], axis=mybir.AxisListType.X, negate=True)
            ex = sb.tile([128, B], F32, tag="ex")
            ssum = sb.tile([128, 1], F32, tag="ssum")
            nc.scalar.activation(
                ex[:], ps_s[:], mybir.ActivationFunctionType.Exp,
                bias=mx[:], accum_out=ssum[:],
            )
            rs = sb.tile([128, 1], F32, tag="rs")
            nc.vector.reciprocal(rs[:], ssum[:])
            sm = sb.tile([128, B], F32, tag="sm")
            nc.vector.tensor_scalar_mul(sm[:], ex[:], rs[:])
            nc.sync.dma_start(out=out_d[:], in_=sm[:])

    nc.compile()
    return nc


def _prep_encoder_inputs(tok, emb16, Wih, Whh, bih, bhh):
    """Per-encoder host prep, all weights pre-scaled by WS."""
    import ml_dtypes
    # wih: [E+1, 12, 128]; row E = folded bias (bih+bhh for r,z; bih for n)
    WihT = Wih.T.astype(np.float32) * WS  # [E, 3H]
    brow = np.concatenate([
        (bih[: 2 * H] + bhh[: 2 * H]),
        bih[2 * H :],
    ]).astype(np.float32) * WS  # [3H]
    wih = np.concatenate([WihT, brow[None, :]], axis=0)  # [E+1, 3H]
    wih = np.ascontiguousarray(wih.reshape(E + 1, 12, 128)).astype(np.float16)
    # whh8: [128(kp), 2(pair j), 2(group a), 12(gate-chunk), 128(m)]
    # pair j, group a <-> k-chunk (2a + j); m-chunk gc of gates r,z,n
    WhhT = Whh.T.astype(np.float32) * WS  # [H, 3H]
    whh8 = np.empty((128, 2, 2, 12, 128), np.float32)
    for j in range(2):
        for a in range(2):
            kc = 2 * a + j
            whh8[:, j, a, :, :] = WhhT[kc * 128 : (kc + 1) * 128, :].reshape(
                128, 12, 128)
    whh8 = whh8.astype(ml_dtypes.float8_e4m3)
    # bhhn: [128, 4] f32, column c = bhh_n chunk c, scaled
    bhhn = np.ascontiguousarray(
        (bhh[2 * H :].astype(np.float32) * WS).reshape(4, 128).T)
    return wih, whh8, bhhn


def _prep_idx(tok_shard):
    """tok_shard [NB, L] -> idx [128, 2L] int32: idx[p, 2s+h] = tok[h*128+p, L-1-s]."""
    t = tok_shard.reshape(2, 128, L)          # [h, p, l]
    rev = t[:, :, ::-1]                        # l -> step s
    idx = rev.transpose(1, 2, 0).reshape(128, L * 2)  # [p, (s, h)]
    return np.ascontiguousarray(idx).astype(np.int32)


_CACHE = {}
TRACE = False
LAST_EXEC_NS = {}


def kernel(contexts, replies, ctx_emb, ctx_Wih, ctx_Whh, ctx_bih, ctx_bhh,
           rep_emb, rep_Wih, rep_Whh, rep_bih, rep_bhh):
    contexts = np.asarray(contexts).astype(np.int32)
    replies = np.asarray(replies).astype(np.int32)
    as32 = lambda a: np.asarray(a, dtype=np.float32)
    ctx_emb16 = as32(ctx_emb).astype(np.float16)
    rep_emb16 = as32(rep_emb).astype(np.float16)

    if "enc" not in _CACHE:
        _CACHE["enc"] = build_encoder()
    if "sco" not in _CACHE:
        _CACHE["sco"] = build_scores()
    enc = _CACHE["enc"]
    sco = _CACHE["sco"]

    cw = _prep_encoder_inputs(contexts, ctx_emb16, as32(ctx_Wih), as32(ctx_Whh),
                              as32(ctx_bih), as32(ctx_bhh))
    rw = _prep_encoder_inputs(replies, rep_emb16, as32(rep_Wih), as32(rep_Whh),
                              as32(rep_bih), as32(rep_bhh))

    in_maps = []
    for c in range(8):
        if c < 4:
            tok, emb16, (wih, whh, bhhn) = contexts, ctx_emb16, cw
            sh = c
        else:
            tok, emb16, (wih, whh, bhhn) = replies, rep_emb16, rw
            sh = c - 4
        in_maps.append({
            "emb": emb16,
            "idx": _prep_idx(tok[sh * NB : (sh + 1) * NB]),
            "wih": wih,
            "whh": whh,
            "bhhn": bhhn,
        })

    res = run_bass_kernel_spmd(enc, in_maps, core_ids=list(range(8)), trace=TRACE)
    if TRACE:
        LAST_EXEC_NS["enc"] = res.exec_time_ns
    houts = [r["hout"] for r in res.results]  # each [128, HC, NB] fp16

    hcT = np.concatenate([houts[c].transpose(1, 0, 2) for c in range(4)], axis=2)
    hrT = np.concatenate([houts[c].transpose(1, 0, 2) for c in range(4, 8)], axis=2)

    in_maps2 = []
    for c in range(8):
        in_maps2.append({
            "hc": np.ascontiguousarray(hcT[:, :, c * 128 : (c + 1) * 128]),
            "hr": np.ascontiguousarray(hrT),
        })
    res2 = run_bass_kernel_spmd(sco, in_maps2, core_ids=list(range(8)), trace=TRACE)
    if TRACE:
        LAST_EXEC_NS["sco"] = res2.exec_time_ns
    out = np.concatenate([r["out"] for r in res2.results], axis=0)
    return out.astype(np.float32)


# revision 6
# speedup vs baseline: 1.2784x; 1.2784x over previous
"""DSSM (dual GRU encoder + BxB softmax similarity) on 8 Trainium2 NeuronCores.

Strategy:
  - Cores 0-3 run the context encoder on batch rows [256c, 256(c+1));
    cores 4-7 run the reply encoder on rows [256(c-4), 256(c-3)).
    One SPMD program; per-core in_maps carry the right table/weights/indices.
  - Encoder: backward GRU over L=128 steps, hidden state kept transposed
    (features on partitions): h as [128 part, 4 chunk, 256 batch] fp16.
    Per step: 66 fp16 matmuls (gi K=101 incl. folded biases via ones column,
    gh K=128, bhh_n via K=1 ones outer product) accumulate gate
    pre-activations in PSUM; ACT does sigmoid/tanh; DVE combines.
    Embedding rows arrive via per-step indirect-DMA gathers (+ PE transpose).
  - Phase 2 (second small kernel): each core computes a 128-row slice of
    scores = hc @ hr.T (fp16 matmuls, fp32 accum) + row softmax.

All tensor layout prep (transposes, bias folding, sharding, time reversal)
is host-side numpy; the FLOP-carrying work runs on device.
"""

import numpy as np

import concourse.bass as bass
import concourse.mybir as mybir
import concourse.tile as tile
from concourse import bacc
from concourse.bass_utils import run_bass_kernel_spmd
from concourse.masks import make_identity

F16 = mybir.dt.float16
F32 = mybir.dt.float32
I32 = mybir.dt.int32

V, E, H, B, L = 50000, 100, 512, 1024, 128
NB = 256          # batch rows per core
HC = H // 128     # 4 h chunks
GA = 6            # gather lookahead (steps)
XA = 2            # xt lookahead


def build_encoder(l_steps=L, loop_n=None, pe_only=False):
    """loop_n: benchmark-only mode — wraps the step loop in a For_i that
    re-runs the whole sequence loop_n times (data goes stale; timing only)."""
    nc = bacc.Bacc("TRN2", target_bir_lowering=False, debug=False)
    emb_d = nc.dram_tensor("emb", [V, E], F16, kind="ExternalInput")
    idx_d = nc.dram_tensor("idx", [128, 2 * l_steps], I32, kind="ExternalInput")
    wih_d = nc.dram_tensor("wih", [E + 1, 12, 128], F16, kind="ExternalInput")
    whh_d = nc.dram_tensor("whh", [128, HC, 3 * H], F16, kind="ExternalInput")
    bhhn_d = nc.dram_tensor("bhhn", [128, 4], F32, kind="ExternalInput")
    hout_d = nc.dram_tensor("hout", [128, HC, NB], F16, kind="ExternalOutput")

    with tile.TileContext(nc) as tc:
        with (
            tc.tile_pool(name="wt", bufs=1) as wt,
            tc.tile_pool(name="grng", bufs=GA + 2) as grng,
            tc.tile_pool(name="xrng", bufs=XA + 2) as xrng,
            tc.tile_pool(name="hrng", bufs=3) as hrng,
            tc.tile_pool(name="gt", bufs=4) as gt,
            tc.tile_pool(name="psr", bufs=1, space="PSUM") as psr,
            tc.tile_pool(name="psz", bufs=1, space="PSUM") as psz,
            tc.tile_pool(name="psghn", bufs=1, space="PSUM") as psghn,
            tc.tile_pool(name="psgin", bufs=2, space="PSUM") as psgin,
        ):
            # --- constants / weights ---
            wih_t = wt.tile([E + 1, 12, 128], F16, tag="wih")
            nc.sync.dma_start(out=wih_t[:], in_=wih_d[:])
            whh_t = wt.tile([128, HC, 3 * H], F16, tag="whh")
            nc.sync.dma_start(out=whh_t[:], in_=whh_d[:])
            bhhn_t = wt.tile([128, 4], F32, tag="bhhn")
            nc.sync.dma_start(out=bhhn_t[:], in_=bhhn_d[:])
            idx_t = wt.tile([128, 2 * l_steps], I32, tag="idx")
            nc.sync.dma_start(out=idx_t[:], in_=idx_d[:])

            # --- rings ---
            n_g = GA + 2
            g_ring = []
            for i in range(n_g):
                g = grng.tile([128, 2, 128], F16, tag="g", name=f"g{i}")
                nc.vector.memset(g[:, :, E:], 0.0)
                nc.vector.memset(g[:, :, E : E + 1], 1.0)
                g_ring.append(g)
            n_x = XA + 2
            xt_ring = [xrng.tile([128, 2, 128], F16, tag="xt", name=f"xt{i}") for i in range(n_x)]
            if pe_only:
                for x in xt_ring:
                    nc.vector.memset(x[:], 0.0)
            h_ring = []
            for i in range(3):
                h = hrng.tile([128, HC, NB], F16, tag="h", name=f"h{i}")
                nc.vector.memset(h[:], 0.0)
                h_ring.append(h)

            def emit_gather(s):
                g = g_ring[s % n_g]
                for hh in range(2):
                    nc.gpsimd.indirect_dma_start(
                        out=g[:, hh, :E],
                        out_offset=None,
                        in_=emb_d[:],
                        in_offset=bass.IndirectOffsetOnAxis(
                            ap=idx_t[:, 2 * s + hh : 2 * s + hh + 1], axis=0
                        ),
                    )

            def emit_xt(s):
                g = g_ring[s % n_g]
                xt = xt_ring[s % n_x]
                for hh in range(2):
                    nc.sync.dma_start_transpose(xt[:, hh, :], g[:, hh, :])

            # prologue
            for s in range(min(GA + 1, l_steps)):
                emit_gather(s)
            emit_xt(0)

            def gh_mms(reg, mi, h_old, start, stop_on_last):
                for kj in range(HC):
                    nc.tensor.matmul(
                        reg,
                        whh_t[:, kj, mi * 128 : (mi + 1) * 128],
                        h_old[:, kj, :],
                        start=(start and kj == 0),
                        stop=(stop_on_last and kj == HC - 1),
                    )

            # chunk pairs: phase 0 handles chunks (0,2), phase 1 chunks (1,3).
            # kj-major matmul order [0,2,1,3] matches the order h chunks are
            # produced by the previous step's tail, so the PE never waits for
            # the full h vector — only for the chunk its current MM reads.
            # PSUM region order is phase-major [c0, c2, c1, c3]: each phase
            # owns whole banks, with ONE accumulation group per bank (start
            # clears has_written bank-wide once; later first-writes to other
            # regions overwrite because their bits are cleared too).
            KJO = [0, 2, 1, 3]
            SLOT = {0: 0, 2: 1, 1: 2, 3: 3}

            def hpair(h, b):
                return h.rearrange("p (a b) x -> p a b x", a=2, b=2)[:, :, b, :]

            from contextlib import nullcontext
            loop_cm = tc.For_i(0, loop_n, 1) if loop_n else nullcontext()
            with loop_cm:
              for s in range(l_steps):
                  h_old = h_ring[s % 3]
                  h_new = h_ring[(s + 1) % 3]
                  xt = xt_ring[s % n_x].rearrange("p a b -> p (a b)")[:101]

                  ps_r = psr.tile([128, 4 * NB], F32, tag="r")
                  ps_z = psz.tile([128, 4 * NB], F32, tag="z")
                  ps_ghn = psghn.tile([128, 4 * NB], F32, tag="ghn")
                  rreg = lambda c: ps_r[:, SLOT[c] * NB : (SLOT[c] + 1) * NB]
                  nreg = lambda c: ps_ghn[:, SLOT[c] * NB : (SLOT[c] + 1) * NB]
                  zreg = lambda c: ps_z[:, SLOT[c] * NB : (SLOT[c] + 1) * NB]
                  r_s = gt.tile([128, 4, NB], F16, tag="r_s")
                  n_s = gt.tile([128, 4, NB], F16, tag="n_s")
                  hmn_s = gt.tile([128, 4, NB], F16, tag="hmn_s")
                  z_s = gt.tile([128, 4, NB], F16, tag="z_s")
                  t_s = gt.tile([128, 4, NB], F16, tag="t_s")

                  # ---- h-independent head (PSUM rule: chains sharing a bank
                  # must be sequential — start=True clears has_written bank-wide;
                  # each phase touches each bank at most once).
                  if not pe_only:
                      if loop_n:
                          emit_xt((s + 1) % l_steps)
                          emit_gather((s + GA + 1) % l_steps)
                      else:
                          if s + 1 < l_steps:
                              emit_xt(s + 1)
                          if s + GA + 1 < l_steps:
                              emit_gather(s + GA + 1)
                  gins = {}

                  def phase_head(ph):
                      ca, cb = (0, 2) if ph == 0 else (1, 3)
                      # one group per bank: start=True only on the bank's first MM
                      nc.tensor.matmul(rreg(ca), wih_t[:, ca, :], xt, start=True, stop=False)
                      nc.tensor.matmul(rreg(cb), wih_t[:, cb, :], xt, start=False, stop=False)

                  def gin_mms(ph):
                      ca, cb = (0, 2) if ph == 0 else (1, 3)
                      ps_gin = psgin.tile([128, 2 * NB], F32, tag="gin", name=f"gin{ph}_{s}")
                      gins[ph] = ps_gin
                      nc.tensor.matmul(ps_gin[:, :NB], wih_t[:, 8 + ca, :], xt,
                                       start=True, stop=True)
                      nc.tensor.matmul(ps_gin[:, NB:], wih_t[:, 8 + cb, :], xt,
                                       start=True, stop=True)

                  def phase_mms(ph):
                      ca, cb = (0, 2) if ph == 0 else (1, 3)
                      for kj in KJO:
                          last = kj == KJO[-1]
                          for c in (ca, cb):
                              nc.tensor.matmul(
                                  rreg(c), whh_t[:, kj, c * 128 : (c + 1) * 128],
                                  h_old[:, kj, :], start=False, stop=(last and c == cb))
                          for c in (ca, cb):
                              nc.tensor.matmul(
                                  nreg(c), whh_t[:, kj, (8 + c) * 128 : (9 + c) * 128],
                                  h_old[:, kj, :], start=(kj == KJO[0] and c == ca),
                                  stop=(last and c == cb))

                  def phase_gates(ph):
                      sl = slice(2 * ph, 2 * ph + 2)
                      psl = slice(2 * ph * NB, (2 * ph + 2) * NB)
                      nc.scalar.activation(r_s[:, sl, :], ps_r[:, psl],
                                           mybir.ActivationFunctionType.Sigmoid)
                      rm = gt.tile([128, 2 * NB], F32, tag="rm")
                      for i, c in enumerate((2 * ph, 2 * ph + 1) if False else ((0, 2) if ph == 0 else (1, 3))):
                          nc.vector.scalar_tensor_tensor(
                              rm[:, i * NB : (i + 1) * NB],
                              ps_ghn[:, (2 * ph + i) * NB : (2 * ph + i + 1) * NB],
                              bhhn_t[:, c : c + 1],
                              r_s[:, 2 * ph + i, :],
                              op0=mybir.AluOpType.add, op1=mybir.AluOpType.mult)
                      pre = gt.tile([128, 2 * NB], F32, tag="pre")
                      nc.vector.tensor_add(pre[:], rm[:], gins[ph][:])
                      nc.scalar.activation(n_s[:, sl, :], pre[:],
                                           mybir.ActivationFunctionType.Tanh)
                      nc.vector.tensor_sub(hmn_s[:, sl, :], hpair(h_old, ph), n_s[:, sl, :])

                  phase_head(0)
                  gin_mms(0)
                  phase_mms(0)
                  phase_head(1)
                  if not pe_only:
                      phase_gates(0)
                  phase_mms(1)
                  gin_mms(1)
                  if not pe_only:
                      phase_gates(1)

                  # z phases: same pairing; per-chunk tail in order (0,2,1,3)
                  for ph, (ca, cb) in enumerate([(0, 2), (1, 3)]):
                      nc.tensor.matmul(zreg(ca), wih_t[:, 4 + ca, :], xt, start=True, stop=False)
                      nc.tensor.matmul(zreg(cb), wih_t[:, 4 + cb, :], xt, start=False, stop=False)
                      for kj in KJO:
                          last = kj == KJO[-1]
                          for c in (ca, cb):
                              nc.tensor.matmul(
                                  zreg(c), whh_t[:, kj, (4 + c) * 128 : (5 + c) * 128],
                                  h_old[:, kj, :], start=False, stop=(last and c == cb))
                      if pe_only:
                          continue
                      for c in (ca, cb):
                          k = SLOT[c]
                          nc.scalar.activation(z_s[:, k : k + 1, :], zreg(c),
                                               mybir.ActivationFunctionType.Sigmoid)
                          nc.vector.tensor_mul(t_s[:, k : k + 1, :], z_s[:, k : k + 1, :],
                                               hmn_s[:, k : k + 1, :])
                          nc.vector.tensor_add(h_new[:, c : c + 1, :], n_s[:, k : k + 1, :],
                                               t_s[:, k : k + 1, :])
                  if pe_only:
                      anchor = gt.tile([128, 4], F32, tag="anchor")
                      nc.vector.tensor_copy(anchor[:, 0:1], ps_r[:, 0:1])
                      nc.vector.tensor_copy(anchor[:, 1:2], ps_z[:, 0:1])
                      nc.vector.tensor_copy(anchor[:, 2:3], ps_ghn[:, 0:1])
                      nc.vector.tensor_copy(anchor[:, 3:4], gins[1][:, 0:1])

            nc.sync.dma_start(out=hout_d[:], in_=h_ring[l_steps % 3][:])

    nc.compile()
    return nc


def build_scores():
    nc = bacc.Bacc("TRN2", target_bir_lowering=False, debug=False)
    hc_d = nc.dram_tensor("hc", [HC, 128, 128], F16, kind="ExternalInput")
    hr_d = nc.dram_tensor("hr", [HC, 128, B], F16, kind="ExternalInput")
    out_d = nc.dram_tensor("out", [128, B], F32, kind="ExternalOutput")

    with tile.TileContext(nc) as tc:
        with (
            tc.tile_pool(name="sb", bufs=1) as sb,
            tc.tile_pool(name="ps", bufs=1, space="PSUM") as ps,
        ):
            hc_t = sb.tile([128, HC, 128], F16, tag="hc")
            hr_t = sb.tile([128, HC, B], F16, tag="hr")
            nc.sync.dma_start(out=hc_t[:], in_=hc_d.rearrange("k p m -> p k m"))
            nc.sync.dma_start(out=hr_t[:], in_=hr_d.rearrange("k p n -> p k n"))
            ps_s = ps.tile([128, B], F32, tag="s")
            for nh in range(2):
                for kj in range(HC):
                    nc.tensor.matmul(
                        ps_s[:, nh * 512 : (nh + 1) * 512],
                        hc_t[:, kj, :],
                        hr_t[:, kj, nh * 512 : (nh + 1) * 512],
                        start=(kj == 0), stop=(kj == HC - 1),
                    )
            mx = sb.tile([128, 1], F32, tag="mx")
            nc.vector.reduce_max(mx[:], ps_s[:], axis=mybir.AxisListType.X, negate=True)
            ex = sb.tile([128, B], F32, tag="ex")
            ssum = sb.tile([128, 1], F32, tag="ssum")
            nc.scalar.activation(
                ex[:], ps_s[:], mybir.ActivationFunctionType.Exp,
                bias=mx[:], accum_out=ssum[:],
            )
            rs = sb.tile([128, 1], F32, tag="rs")
            nc.vector.reciprocal(rs[:], ssum[:])
            sm = sb.tile([128, B], F32, tag="sm")
            nc.vector.tensor_scalar_mul(sm[:], ex[:], rs[:])
            nc.sync.dma_start(out=out_d[:], in_=sm[:])

    nc.compile()
    return nc


def _prep_encoder_inputs(tok, emb16, Wih, Whh, bih, bhh):
    """Per-encoder host prep. tok [B, L] int; returns dict pieces shared by its 4 cores."""
    # wih: [E+1, 12, 128]; row E = folded bias (bih+bhh for r,z; bih for n)
    WihT = Wih.T.astype(np.float32)  # [E, 3H]
    brow = np.concatenate([
        (bih[: 2 * H] + bhh[: 2 * H]),
        bih[2 * H :],
    ]).astype(np.float32)  # [3H]
    wih = np.concatenate([WihT, brow[None, :]], axis=0)  # [E+1, 3H]
    wih = np.ascontiguousarray(
        wih.reshape(E + 1, 12, 128)
    ).astype(np.float16)
    # whh: [128, HC, 3H]: whh[p, kj, m] = Whh.T[kj*128+p, m] = Whh[m, kj*128+p]
    whh = np.ascontiguousarray(
        Whh.T.astype(np.float32).reshape(HC, 128, 3 * H).transpose(1, 0, 2)
    ).astype(np.float16)
    bhhn = np.ascontiguousarray(
        bhh[2 * H :].astype(np.float32).reshape(4, 128).T)  # [128, 4]
    return wih, whh, bhhn


def _prep_idx(tok_shard):
    """tok_shard [NB, L] -> idx [128, 2L] int32: idx[p, 2s+h] = tok[h*128+p, L-1-s]."""
    t = tok_shard.reshape(2, 128, L)          # [h, p, l]
    rev = t[:, :, ::-1]                        # l -> step s
    idx = rev.transpose(1, 2, 0).reshape(128, L * 2)  # [p, (s, h)]
    return np.ascontiguousarray(idx).astype(np.int32)


_CACHE = {}
TRACE = False
LAST_EXEC_NS = {}


def kernel(contexts, replies, ctx_emb, ctx_Wih, ctx_Whh, ctx_bih, ctx_bhh,
           rep_emb, rep_Wih, rep_Whh, rep_bih, rep_bhh):
    contexts = np.asarray(contexts).astype(np.int32)
    replies = np.asarray(replies).astype(np.int32)
    as32 = lambda a: np.asarray(a, dtype=np.float32)
    ctx_emb16 = as32(ctx_emb).astype(np.float16)
    rep_emb16 = as32(rep_emb).astype(np.float16)

    if "enc" not in _CACHE:
        _CACHE["enc"] = build_encoder()
    if "sco" not in _CACHE:
        _CACHE["sco"] = build_scores()
    enc = _CACHE["enc"]
    sco = _CACHE["sco"]

    cw = _prep_encoder_inputs(contexts, ctx_emb16, as32(ctx_Wih), as32(ctx_Whh),
                              as32(ctx_bih), as32(ctx_bhh))
    rw = _prep_encoder_inputs(replies, rep_emb16, as32(rep_Wih), as32(rep_Whh),
                              as32(rep_bih), as32(rep_bhh))

    in_maps = []
    for c in range(8):
        if c < 4:
            tok, emb16, (wih, whh, bhhn) = contexts, ctx_emb16, cw
            sh = c
        else:
            tok, emb16, (wih, whh, bhhn) = replies, rep_emb16, rw
            sh = c - 4
        in_maps.append({
            "emb": emb16,
            "idx": _prep_idx(tok[sh * NB : (sh + 1) * NB]),
            "wih": wih,
            "whh": whh,
            "bhhn": bhhn,
        })

    res = run_bass_kernel_spmd(enc, in_maps, core_ids=list(range(8)), trace=TRACE)
    if TRACE:
        LAST_EXEC_NS["enc"] = res.exec_time_ns
    houts = [r["hout"] for r in res.results]  # each [128, HC, NB] fp16

    # assemble hcT_all / hrT_all: [HC, 128, B] fp16 (feature-chunked, batch on free)
    hcT = np.concatenate([houts[c].transpose(1, 0, 2) for c in range(4)], axis=2)
    hrT = np.concatenate([houts[c].transpose(1, 0, 2) for c in range(4, 8)], axis=2)

    in_maps2 = []
    for c in range(8):
        in_maps2.append({
            "hc": np.ascontiguousarray(hcT[:, :, c * 128 : (c + 1) * 128]),
            "hr": np.ascontiguousarray(hrT),
        })
    res2 = run_bass_kernel_spmd(sco, in_maps2, core_ids=list(range(8)), trace=TRACE)
    if TRACE:
        LAST_EXEC_NS["sco"] = res2.exec_time_ns
    out = np.concatenate([r["out"] for r in res2.results], axis=0)
    return out.astype(np.float32)

